# revision 1
# baseline (speedup 1.0000x reference)
"""Trainium2 8-core Bass kernel for nn_Atom_Inter_Layer (GNN attention message passing).

Strategy (see sharding hint): edges are sharded by destination-node range.
Core c owns nodes [1250*c, 1250*(c+1)) and every edge whose dst lands there,
so segment-softmax/sum are fully core-local: no collectives. Host does
index bucketing/padding + weight folding only; all NN compute runs on device.

Algebraic restructuring (validated in algo_check.py, rel err ~4.5e-3 bf16):
  x_feat = LN(concat(ea, x[src], x[dst])) feeding k/v MLPs is decomposed as
    Q_e = ea_e @ wEA' + (sea_e/576)*(-v1) + A'[src_e] + B'[dst_e]
  where A'/B' are per-node tables computed on device (x @ folded W1 parts,
  minus the LN-mean term), and the LN's 1/std factor is dropped entirely
  because the following mid-LayerNorm is scale-invariant per row (exact up
  to the eps term; error ~5e-6). LN gamma/beta and all biases fold into the
  tables host-side (they are ones/zeros in this problem; nonzero values are
  still folded exactly, except a nonzero ln_norm_b or k_b1/v_b1 would break
  scale invariance - asserted below).

Per-core device pipeline:
  prep: A' table [10016,768]bf16 -> DRAM; B'/q tables for local nodes -> SBUF
  main: for each 125-node block (10 per core):
    indirect-DMA gather A'[src] for the block's (padded) edges
    per 128-edge subtile: selector matrix S from dst-vs-iota equality;
    ea/B matmuls + gathered A' -> h1; LayerNorm+SiLU (bn_stats, ACT Silu
    with per-partition scale/bias); PE transposes; second matmuls -> k,v;
    q[dst] gather via S-matmul; scores -> exp -> m_ext=[u*v | u];
    scatter-matmul S.T @ m_ext accumulated in PSUM over the block
    epilogue: alpha-normalize, output MLP, DMA out rows.
"""
import sys

if "/opt/trn_rl_repo" not in sys.path:
    sys.path.insert(0, "/opt/trn_rl_repo")

from contextlib import ExitStack

import numpy as np
import ml_dtypes

import concourse.bass as bass
import concourse.bacc as bacc
import concourse.tile as tile
import bass_rust as _bass_rust
from concourse.hw_specs import get_activation_tables as _gat


def _patched_iatl(self):
    import concourse.mybir as _mb
    has_activation = any(
        isinstance(i, _mb.InstActivation)
        for b in self.main_func.blocks
        for i in b.instructions
    )
    if not has_activation:
        return
    tables = list(_gat(self.m.arch).items())
    if _FILTER_TABLES:
        keep = "natural_log_exp_and_others"
        import concourse.mybir as _mb2
        drop = {_mb2.ActivationFunctionType.Exp, _mb2.ActivationFunctionType.Ln,
                _mb2.ActivationFunctionType.Copy, _mb2.ActivationFunctionType.Identity}
        tables = [(n, (set(fns) if n == keep else {f for f in fns if f not in drop}))
                  for n, fns in tables]
    _bass_rust.insert_act_table_loads(self, tables)


_FILTER_TABLES = True
bacc.Bacc.insert_act_table_loads = _patched_iatl
from concourse import mybir
from concourse.bass_utils import run_bass_kernel_spmd

BF16 = mybir.dt.bfloat16
F32 = mybir.dt.float32
I32 = mybir.dt.int32

N, E, D, EDIM, H, C = 10000, 320000, 256, 64, 8, 32
FEAT = 2 * D + EDIM  # 576
NCORE = 8
NB = 10            # node blocks per core
BLK = 125          # nodes per block
NPC = NB * BLK     # nodes per core = 1250
NT = (N + 127) // 128          # 79 tiles of the full node table
NPAD = NT * 128                # 10112
EPS = 1e-5

bf16 = ml_dtypes.bfloat16


def _b(a):
    return np.ascontiguousarray(np.asarray(a, np.float32)).astype(bf16)


def host_prep(inputs):
    """Build per-core in_maps + static shapes from the full inputs."""
    x = np.asarray(inputs["x"], np.float32)
    ei = np.asarray(inputs["edge_index"]).astype(np.int64)
    ea = np.asarray(inputs["edge_attr"], np.float32)
    src, dst = ei[0], ei[1]
    g = np.asarray(inputs["ln_norm_g"], np.float32)
    b = np.asarray(inputs["ln_norm_b"], np.float32)
    kw1 = np.asarray(inputs["k_w1"], np.float32)
    vw1 = np.asarray(inputs["v_w1"], np.float32)

    # --- fast-path validity (see module docstring) ---
    for nm in ("ln_norm_b", "k_b1", "v_b1", "q_b1", "q_b2", "k_b2", "v_b2",
               "o_b1", "o_b2", "q_be", "k_be", "v_be"):
        assert np.abs(np.asarray(inputs[nm])).max() == 0.0, f"{nm} nonzero; fast path invalid"
    for nm in ("q_g", "k_g", "v_g"):
        assert np.abs(np.asarray(inputs[nm]) - 1.0).max() == 0.0, f"{nm} != 1"

    # --- weight folds (f32 host math) ---
    v1cat = np.concatenate([g @ kw1, g @ vw1])                     # [768]
    rk1 = v1cat[None, :] / FEAT   # rank-1 LN-mean fold, subtracted from every W1 row
    wEAx = np.concatenate([g[:64, None] * kw1[0:64], g[:64, None] * vw1[0:64]], 1) - rk1  # [64,768]
    wA = np.concatenate([g[64:320, None] * kw1[64:320], g[64:320, None] * vw1[64:320]], 1) - rk1
    wB = np.concatenate([g[320:576, None] * kw1[320:576], g[320:576, None] * vw1[320:576]], 1) - rk1

    # --- edge bucketing by destination block ---
    bucket = (dst // BLK).astype(np.int64)           # 0..79
    order = np.argsort(bucket, kind="stable")
    counts = np.bincount(bucket, minlength=NCORE * NB)
    nsub = int(np.ceil(counts.max() / 128))
    tblk = nsub * 128
    starts = np.zeros(NCORE * NB, np.int64)
    starts[1:] = np.cumsum(counts)[:-1]
    pos_in_blk = np.arange(E, dtype=np.int64) - starts[bucket[order]]  # position within its block

    # padded per-edge arrays, laid out [80, tblk] with edge (blk, t*128+p)
    idx_pad = np.zeros((NCORE * NB, tblk), np.int32)          # src gather index (pad -> 0)
    dst_pad = np.full((NCORE * NB, tblk), -1.0, np.float32)   # block-local dst (pad -> -1)
    ea_pad = np.zeros((NCORE * NB, tblk, EDIM), np.float32)
    bo = bucket[order]
    idx_pad[bo, pos_in_blk] = src[order].astype(np.int32)
    dst_pad[bo, pos_in_blk] = (dst[order] - bo * BLK).astype(np.float32)
    ea_pad[bo, pos_in_blk, :] = ea[order]

    # device layouts
    # eaT: [core][64, NB*tblk], column order (block, t, p)
    eaT = ea_pad.reshape(NCORE, NB * tblk, EDIM).transpose(0, 2, 1)
    # dst: [core][NB, 128, nsub] with [b, p, t] = edge (b, t*128+p)
    dst_c = dst_pad.reshape(NCORE, NB, nsub, 128).transpose(0, 1, 3, 2)
    # dma_gather int16 indices: idx i at [i%16, i//16], replicated to 128 partitions
    idx16 = idx_pad.astype(np.int16).reshape(NCORE, NB, tblk // 16, 16).transpose(0, 1, 3, 2)
    idx16 = np.broadcast_to(idx16[:, :, None, :, :], (NCORE, NB, 8, 16, tblk // 16))
    idx16 = np.ascontiguousarray(idx16).reshape(NCORE, NB, 128, tblk // 16)

    # node-table layouts
    xpad = np.zeros((NPAD, D), np.float32)
    xpad[:N] = x
    xTfull = _b(xpad.T.reshape(D, NPAD))                    # [256, 10112]
    xTblk = x.reshape(NCORE, NB, BLK, D)                    # block-padded local xT [core][256, NB*128]
    xTb = np.zeros((NCORE, D, NB, 128), np.float32)
    xTb[:, :, :, :BLK] = xTblk.transpose(0, 3, 1, 2)

    iotam = np.tile(np.concatenate([np.arange(BLK, dtype=np.float32),
                                    np.full(128 - BLK, -2.0, np.float32)]), (128, 1))
    ident = np.eye(128, dtype=np.float32)

    shapes = dict(nsub=nsub, tblk=tblk)
    common = {
        "eaT": None,  # per-core below
        "wEAx": _b(wEAx),
        "wA": _b(wA.reshape(2, 128, 768)),
        "wB": _b(wB.reshape(2, 128, 768)),
        "wq1": _b(np.asarray(inputs["q_w1"], np.float32).reshape(2, 128, 512)),
        "wq2": _b(np.asarray(inputs["q_w2"], np.float32).reshape(4, 128, 256)),
        "w2k": _b(np.asarray(inputs["k_w2"], np.float32).reshape(2, 128, 256)),
        "w2v": _b(np.asarray(inputs["v_w2"], np.float32).reshape(4, 128, 256)),
        "wo1": _b(np.asarray(inputs["o_w1"], np.float32).reshape(2, 128, 512)),
        "wo2": _b(np.asarray(inputs["o_w2"], np.float32).reshape(4, 128, 256)),
        "iotam": _b(iotam),
        "ident": _b(ident),
        "xTfull": xTfull,
    }
    in_maps = []
    for c in range(NCORE):
        m = dict(common)
        m["eaT"] = _b(eaT[c])
        m["idx"] = np.ascontiguousarray(idx16[c])
        m["dstl"] = _b(dst_c[c])
        m["xTb"] = _b(xTb[c].reshape(D, NB * 128))
        in_maps.append(m)
    return in_maps, shapes


def build(nsub, tblk, debug=False, finalize=True):
    """Build the single-core Bass graph (same on all 8 cores)."""
    nc = bacc.Bacc()
    p_eaT = nc.declare_dram_parameter("eaT", [EDIM, NB * tblk], BF16, isOutput=False)
    p_idx = nc.declare_dram_parameter("idx", [NB, 128, tblk // 16], mybir.dt.int16, isOutput=False)
    p_dst = nc.declare_dram_parameter("dstl", [NB, 128, nsub], BF16, isOutput=False)
    p_xTb = nc.declare_dram_parameter("xTb", [D, NB * 128], BF16, isOutput=False)
    p_xTf = nc.declare_dram_parameter("xTfull", [D, NPAD], BF16, isOutput=False)
    p_wEAx = nc.declare_dram_parameter("wEAx", [EDIM, 768], BF16, isOutput=False)
    p_wA = nc.declare_dram_parameter("wA", [2, 128, 768], BF16, isOutput=False)
    p_wB = nc.declare_dram_parameter("wB", [2, 128, 768], BF16, isOutput=False)
    p_wq1 = nc.declare_dram_parameter("wq1", [2, 128, 512], BF16, isOutput=False)
    p_wq2 = nc.declare_dram_parameter("wq2", [4, 128, 256], BF16, isOutput=False)
    p_w2k = nc.declare_dram_parameter("w2k", [2, 128, 256], BF16, isOutput=False)
    p_w2v = nc.declare_dram_parameter("w2v", [4, 128, 256], BF16, isOutput=False)
    p_wo1 = nc.declare_dram_parameter("wo1", [2, 128, 512], BF16, isOutput=False)
    p_wo2 = nc.declare_dram_parameter("wo2", [4, 128, 256], BF16, isOutput=False)
    p_iotam = nc.declare_dram_parameter("iotam", [128, 128], BF16, isOutput=False)
    p_ident = nc.declare_dram_parameter("ident", [128, 128], BF16, isOutput=False)
    p_out = nc.declare_dram_parameter("out", [NPC, D], F32, isOutput=True)
    p_dbg = nc.declare_dram_parameter("dbg", [8, 128, 768], F32, isOutput=True) if debug else None
    A_dram = nc.dram_tensor("A_tab", [NPAD, 768], BF16)

    isq32 = 1.0 / np.sqrt(C)

    with tile.TileContext(nc) as tc, ExitStack() as ctx:
        const = ctx.enter_context(tc.tile_pool(name="const", bufs=1))
        persist = ctx.enter_context(tc.tile_pool(name="persist", bufs=1))
        # psum pools
        pp_tp = ctx.enter_context(tc.tile_pool(name="pp_tp", bufs=1, space="PSUM"))
        pp_k = ctx.enter_context(tc.tile_pool(name="pp_k", bufs=2, space="PSUM"))
        pp_v = ctx.enter_context(tc.tile_pool(name="pp_v", bufs=2, space="PSUM"))
        pp_q = ctx.enter_context(tc.tile_pool(name="pp_q", bufs=1, space="PSUM"))
        pp_kv = ctx.enter_context(tc.tile_pool(name="pp_kv", bufs=1, space="PSUM"))
        pp_acc = ctx.enter_context(tc.tile_pool(name="pp_acc", bufs=1, space="PSUM"))
        # sbuf working pools
        sp_g = ctx.enter_context(tc.tile_pool(name="sp_gath", bufs=2))
        sp_ea = ctx.enter_context(tc.tile_pool(name="sp_ea", bufs=2))
        sp_w = ctx.enter_context(tc.tile_pool(name="sp_w", bufs=3))
        sp_s = ctx.enter_context(tc.tile_pool(name="sp_s", bufs=2))
        sp_sm = ctx.enter_context(tc.tile_pool(name="sp_sm", bufs=1))
        sp_t = ctx.enter_context(tc.tile_pool(name="sp_small", bufs=4))
        sp_o = ctx.enter_context(tc.tile_pool(name="sp_out", bufs=2))

        # ---- constants to SBUF ----
        def cload(param, shape, dtype=BF16, rearr=None, **rkw):
            t = const.tile(shape, dtype, tag=param.name)
            src = param[:]
            if rearr:
                src = src.rearrange(rearr, **rkw)
            nc.sync.dma_start(out=t[:], in_=src)
            return t

        wEAx = cload(p_wEAx, [EDIM, 768])
        wA = cload(p_wA, [128, 2, 768], rearr="j p c -> p j c")
        wB = cload(p_wB, [128, 2, 768], rearr="j p c -> p j c")
        wq1 = cload(p_wq1, [128, 2, 512], rearr="j p c -> p j c")
        wq2 = cload(p_wq2, [128, 4, 256], rearr="j p c -> p j c")
        w2k = cload(p_w2k, [128, 2, 256], rearr="j p c -> p j c")
        w2v = cload(p_w2v, [128, 4, 256], rearr="j p c -> p j c")
        wo1 = cload(p_wo1, [128, 2, 512], rearr="j p c -> p j c")
        wo2 = cload(p_wo2, [128, 4, 256], rearr="j p c -> p j c")
        iotam = cload(p_iotam, [128, 128])
        ident = cload(p_ident, [128, 128])
        xTb = cload(p_xTb, [128, 2, NB * 128], rearr="(j p) n -> p j n", p=128)

        epsc = const.tile([128, 1], F32)
        nc.vector.memset(epsc[:], EPS)

        B_sb = persist.tile([128, NB, 768], BF16)
        q_sb = persist.tile([128, NB, 256], BF16)

        # ================= PREP =================
        def table_tile(lhs_sb, lhs_col0, wghts, a_out):
            """one 128-node tile of A'/B' table: x @ W_folded."""
            pk = pp_k.tile([128, 256], F32, tag="pk")
            pv = pp_v.tile([128, 512], F32, tag="pv")
            for j in range(2):
                lhsT = lhs_sb[:, j, lhs_col0:lhs_col0 + 128]
                nc.tensor.matmul(pk[:], lhsT, wghts[:, j, 0:256], start=(j == 0), stop=(j == 1))
                nc.tensor.matmul(pv[:], lhsT, wghts[:, j, 256:768], start=(j == 0), stop=(j == 1))
            nc.vector.tensor_copy(out=a_out[:, 0:256], in_=pk[:])
            nc.vector.tensor_copy(out=a_out[:, 256:768], in_=pv[:])

        prep_stack = ExitStack()
        prep = prep_stack.enter_context(tc.tile_pool(name="prep", bufs=2))
        prepc = prep_stack.enter_context(tc.tile_pool(name="prepc", bufs=1))
        xTf = prepc.tile([128, 2, NPAD], BF16)
        nc.sync.dma_start(out=xTf[:], in_=p_xTf[:].rearrange("(j p) n -> p j n", p=128))
        for i in range(NT):
            a_out = prep.tile([128, 768], BF16, tag="aout")
            table_tile(xTf, i * 128, wA, a_out)
            nc.gpsimd.dma_start(out=A_dram[i * 128:(i + 1) * 128, 0:256], in_=a_out[:, 0:256])
            nc.gpsimd.dma_start(out=A_dram[i * 128:(i + 1) * 128, 256:768], in_=a_out[:, 256:768])

        # B' and q tables for local blocks
        def mid_ln_silu(h_ap, width, s_out_ap):
            """s_out = Silu(LayerNorm(h)) over free axis (width<=512).
            Only Ln/Exp ACT funcs (single table set): rsqrt = Exp(-0.5*Ln),
            silu(x) = x / (1 + Exp(-x))."""
            st = sp_t.tile([128, 6], F32, tag="bn")
            nc.vector.bn_stats(out=st[:], in_=h_ap)
            mv = sp_t.tile([128, 2], F32, tag="mv")
            nc.vector.bn_aggr(out=mv[:], in_=st[:])
            lv = sp_t.tile([128, 1], F32, tag="lv")
            nc.scalar.activation(out=lv[:], in_=mv[:, 1:2],
                                 func=mybir.ActivationFunctionType.Ln,
                                 bias=epsc[:], scale=1.0)
            rs = sp_t.tile([128, 1], F32, tag="rs")
            nc.scalar.activation(out=rs[:], in_=lv[:],
                                 func=mybir.ActivationFunctionType.Exp,
                                 bias=0.0, scale=-0.5)
            b2 = sp_t.tile([128, 1], F32, tag="b2")
            nc.vector.tensor_scalar(out=b2[:], in0=mv[:, 0:1], scalar1=rs[:],
                                    scalar2=-1.0, op0=mybir.AluOpType.mult,
                                    op1=mybir.AluOpType.mult)
            xn = sp_s.tile([128, 512], F32, tag="xn")
            xa = xn[:, 0:width]
            nc.vector.tensor_scalar(out=xa, in0=h_ap, scalar1=rs[:], scalar2=b2[:],
                                    op0=mybir.AluOpType.mult, op1=mybir.AluOpType.add)
            en = sp_s.tile([128, 512], F32, tag="en")
            ea_ = en[:, 0:width]
            nc.scalar.activation(out=ea_, in_=xa,
                                 func=mybir.ActivationFunctionType.Exp,
                                 bias=0.0, scale=-1.0)
            nc.vector.tensor_scalar_add(out=ea_, in0=ea_, scalar1=1.0)
            nc.vector.reciprocal_approx_fast(out=ea_, in_=ea_)
            nc.vector.tensor_tensor(out=s_out_ap, in0=xa, in1=ea_,
                                    op=mybir.AluOpType.mult)

        for b in range(NB):
            # B'
            bt = sp_w.tile([128, 768], BF16, tag="btab")
            table_tile(xTb, b * 128, wB, bt)
            nc.vector.tensor_copy(out=B_sb[:, b, :], in_=bt[:])
            # q
            pq1 = pp_v.tile([128, 512], F32, tag="pv")
            for j in range(2):
                nc.tensor.matmul(pq1[:], xTb[:, j, b * 128:(b + 1) * 128], wq1[:, j, :],
                                 start=(j == 0), stop=(j == 1))
            sq = sp_s.tile([128, 768], BF16, tag="s")
            mid_ln_silu(pq1[:], 512, sq[:, 0:512])
            sqT = sp_s.tile([128, 6, 128], BF16, tag="sT")
            for i in range(4):
                tp = pp_tp.tile([128, 128], BF16, tag="tp")
                nc.tensor.transpose(tp[:], sq[:, i * 128:(i + 1) * 128], ident[:])
                nc.vector.tensor_copy(out=sqT[:, i, :], in_=tp[:])
            pq2 = pp_q.tile([128, 256], F32, tag="pq")
            for i in range(4):
                nc.tensor.matmul(pq2[:], sqT[:, i, :], wq2[:, i, :],
                                 start=(i == 0), stop=(i == 3))
            nc.scalar.copy(out=q_sb[:, b, :], in_=pq2[:])

        prep_stack.close()
        tc.strict_bb_all_engine_barrier()

        # ================= MAIN =================
        gch = (nsub + 1) // 2
        chunks = [(i * gch, min(nsub, (i + 1) * gch)) for i in range((nsub + gch - 1) // gch)]

        for b in range(NB):
            eaT_t = sp_ea.tile([EDIM, tblk], BF16, tag="ea")
            nc.sync.dma_start(out=eaT_t[:], in_=p_eaT[:, b * tblk:(b + 1) * tblk])
            idx_t = sp_t.tile([128, tblk // 16], mybir.dt.int16, tag="idx")
            nc.sync.dma_start(out=idx_t[:], in_=p_idx[b])
            dst_t = sp_t.tile([128, nsub], BF16, tag="dst")
            nc.sync.dma_start(out=dst_t[:], in_=p_dst[b])

            S_all = sp_sm.tile([128, nsub, 128], BF16, tag="sall")
            Sne_all = sp_sm.tile([128, nsub, 128], BF16, tag="snall")
            for t in range(nsub):
                dcol = dst_t[:, t:t + 1]
                dbc = bass.AP(tensor=dcol.tensor, offset=dcol.offset,
                              ap=[dcol.ap[0], [0, 128]])
                nc.vector.tensor_tensor(out=S_all[:, t, :], in0=dbc, in1=iotam[:],
                                        op=mybir.AluOpType.is_equal)
                stp = pp_tp.tile([128, 128], BF16, tag="tp")
                nc.tensor.transpose(stp[:], S_all[:, t, :], ident[:])
                nc.scalar.copy(out=Sne_all[:, t, :], in_=stp[:])

            acc = pp_acc.tile([128, 264], F32, tag="acc")

            for (h0, h1c) in chunks:
                cnt = h1c - h0
                if cnt <= 0:
                    continue
                ag = sp_g.tile([128, gch, 768], BF16, tag="ag")
                nc.gpsimd.dma_gather(
                    out_ap=ag[:, 0:cnt, :],
                    in_ap=A_dram[:],
                    idxs_ap=idx_t[:, h0 * 8:h1c * 8],
                    num_idxs=cnt * 128,
                    num_idxs_reg=cnt * 128,
                    elem_size=768,
                    single_packet=False,
                )
                for t in range(h0, h1c):
                    j = t - h0
                    S_eb = S_all[:, t, :]
                    S_ne = Sne_all[:, t, :]

                    # layer 1: ea part + B part (+ gathered A); qg in cols 256:512
                    pkt = pp_k.tile([128, 256], F32, tag="pk")
                    pk = pkt[:]
                    pqt = pp_q.tile([128, 256], F32, tag="pq")
                    pq = pqt[:]
                    pv = pp_v.tile([128, 512], F32, tag="pv")
                    ea_l = eaT_t[:, t * 128:(t + 1) * 128]
                    nc.tensor.matmul(pk, ea_l, wEAx[:, 0:256], start=True, stop=False)
                    nc.tensor.matmul(pk, S_ne, B_sb[:, b, 0:256], start=False, stop=True)
                    nc.tensor.matmul(pv[:], ea_l, wEAx[:, 256:768], start=True, stop=False)
                    nc.tensor.matmul(pv[:], S_ne, B_sb[:, b, 256:768], start=False, stop=True)
                    nc.tensor.matmul(pq, S_ne, q_sb[:, b, :], start=True, stop=True)

                    h1 = sp_s.tile([128, 768], BF16, tag="h1")
                    nc.vector.tensor_tensor(out=h1[:, 0:256], in0=pk, in1=ag[:, j, 0:256],
                                            op=mybir.AluOpType.add)
                    nc.vector.tensor_tensor(out=h1[:, 256:768], in0=pv[:], in1=ag[:, j, 256:768],
                                            op=mybir.AluOpType.add)
                    s = sp_s.tile([128, 768], BF16, tag="s")
                    mid_ln_silu(h1[:, 0:256], 256, s[:, 0:256])
                    mid_ln_silu(h1[:, 256:768], 512, s[:, 256:768])

                    sT = sp_s.tile([128, 6, 128], BF16, tag="sT")
                    for i in range(6):
                        tp = pp_tp.tile([128, 128], BF16, tag="tp")
                        nc.tensor.transpose(tp[:], s[:, i * 128:(i + 1) * 128], ident[:])
                        nc.vector.tensor_copy(out=sT[:, i, :], in_=tp[:])

                    kv = pp_kv.tile([128, 512], F32, tag="kv")
                    for i in range(2):
                        nc.tensor.matmul(kv[:, 0:256], sT[:, i, :], w2k[:, i, :],
                                         start=(i == 0), stop=(i == 1))
                    for i in range(4):
                        nc.tensor.matmul(kv[:, 256:512], sT[:, 2 + i, :], w2v[:, i, :],
                                         start=(i == 0), stop=(i == 3))

                    # scores (only one PSUM operand allowed per DVE op)
                    k2s = sp_w.tile([128, 256], BF16, tag="k2s")
                    nc.vector.tensor_copy(out=k2s[:], in_=kv[:, 0:256])
                    prod = sp_w.tile([128, 8, 32], F32, tag="prod")
                    nc.vector.tensor_tensor(out=prod[:], in0=pq.rearrange("p (h c) -> p h c", h=8),
                                            in1=k2s[:].rearrange("p (h c) -> p h c", h=8),
                                            op=mybir.AluOpType.mult)
                    score = sp_t.tile([128, 8], F32, tag="score")
                    nc.vector.tensor_reduce(out=score[:], in_=prod[:],
                                            axis=mybir.AxisListType.X, op=mybir.AluOpType.add)
                    m_ext = sp_w.tile([128, 264], BF16, tag="mext")
                    nc.scalar.activation(out=m_ext[:, 256:264], in_=score[:],
                                         func=mybir.ActivationFunctionType.Exp,
                                         bias=0.0, scale=float(isq32))
                    u = m_ext[:, 256:264]
                    ubc = bass.AP(tensor=u.tensor, offset=u.offset,
                                  ap=[u.ap[0], u.ap[1], [0, 32]])
                    nc.vector.tensor_tensor(out=m_ext[:, 0:256].rearrange("p (h c) -> p h c", h=8),
                                            in0=kv[:, 256:512].rearrange("p (h c) -> p h c", h=8),
                                            in1=ubc, op=mybir.AluOpType.mult)

                    nc.tensor.matmul(acc[:], S_eb, m_ext[:],
                                     start=(t == 0), stop=(t == nsub - 1))

                    if debug and b == 0 and t == 0:
                        nc.gpsimd.dma_start(out=p_dbg[0, :, 0:128], in_=S_eb)
                        nc.gpsimd.dma_start(out=p_dbg[1, :, 0:128], in_=S_ne)
                        dpk = sp_w.tile([128, 256], F32, tag="dk2")
                        nc.vector.tensor_copy(out=dpk[:], in_=pk)
                        nc.gpsimd.dma_start(out=p_dbg[2, :, 0:256], in_=dpk[:])
                        dpv = sp_w.tile([128, 512], F32, tag="dkv")
                        nc.vector.tensor_copy(out=dpv[:], in_=pv[:])
                        nc.gpsimd.dma_start(out=p_dbg[2, :, 256:768], in_=dpv[:, 0:512])
                        nc.gpsimd.dma_start(out=p_dbg[3], in_=s[:])
                        nc.gpsimd.dma_start(out=p_dbg[4, :, 0:264], in_=m_ext[:])
                        nc.gpsimd.dma_start(out=p_dbg[5, :, 0:768], in_=ag[:, j, :])
                        dk2 = sp_w.tile([128, 256], F32, tag="dk2")
                        nc.vector.tensor_copy(out=dk2[:], in_=pq)
                        nc.gpsimd.dma_start(out=p_dbg[6, :, 0:256], in_=dk2[:])
                        dkv = sp_w.tile([128, 512], F32, tag="dkv")
                        nc.vector.tensor_copy(out=dkv[:], in_=kv[:])
                        nc.gpsimd.dma_start(out=p_dbg[7, :, 0:512], in_=dkv[:])
            if debug and b == 0:
                atest = sp_g.tile([128, gch, 768], BF16, tag="ag")
                nc.sync.dma_start(out=atest[:, 0, :], in_=A_dram[0:128, :])
                nc.gpsimd.dma_start(out=p_dbg[6, :, 256:768], in_=atest[:, 0, 0:512])

            # ---- block epilogue: normalize + output MLP ----
            dmx = sp_t.tile([128, 8], F32, tag="dmx")
            nc.vector.tensor_scalar_max(out=dmx[:], in0=acc[:, 256:264], scalar1=1e-30)
            rec = sp_t.tile([128, 8], F32, tag="rec")
            nc.vector.reciprocal(out=rec[:], in_=dmx[:])
            agg = sp_w.tile([128, 256], BF16, tag="agg")
            rap = rec[:]
            rbc = bass.AP(tensor=rap.tensor, offset=rap.offset,
                          ap=[rap.ap[0], rap.ap[1], [0, 32]])
            nc.vector.tensor_tensor(out=agg[:].rearrange("p (h c) -> p h c", h=8),
                                    in0=acc[:, 0:256].rearrange("p (h c) -> p h c", h=8),
                                    in1=rbc, op=mybir.AluOpType.mult)
            aT = sp_s.tile([128, 6, 128], BF16, tag="sT")
            for i in range(2):
                tp = pp_tp.tile([128, 128], BF16, tag="tp")
                nc.tensor.transpose(tp[:], agg[:, i * 128:(i + 1) * 128], ident[:])
                nc.vector.tensor_copy(out=aT[:, i, :], in_=tp[:])
            po1 = pp_v.tile([128, 512], F32, tag="pv")
            for i in range(2):
                nc.tensor.matmul(po1[:], aT[:, i, :], wo1[:, i, :],
                                 start=(i == 0), stop=(i == 1))
            so = sp_s.tile([128, 768], BF16, tag="s")
            eo = sp_s.tile([128, 512], F32, tag="en")
            nc.scalar.activation(out=eo[:], in_=po1[:],
                                 func=mybir.ActivationFunctionType.Exp,
                                 bias=0.0, scale=-1.0)
            nc.vector.tensor_scalar_add(out=eo[:], in0=eo[:], scalar1=1.0)
            nc.vector.reciprocal_approx_fast(out=eo[:], in_=eo[:])
            nc.vector.tensor_tensor(out=so[:, 0:512], in0=po1[:], in1=eo[:],
                                    op=mybir.AluOpType.mult)
            soT = sp_s.tile([128, 6, 128], BF16, tag="sT")
            for i in range(4):
                tp = pp_tp.tile([128, 128], BF16, tag="tp")
                nc.tensor.transpose(tp[:], so[:, i * 128:(i + 1) * 128], ident[:])
                nc.vector.tensor_copy(out=soT[:, i, :], in_=tp[:])
            po2 = pp_q.tile([128, 256], F32, tag="pq")
            for i in range(4):
                nc.tensor.matmul(po2[:], soT[:, i, :], wo2[:, i, :],
                                 start=(i == 0), stop=(i == 3))
            outt = sp_o.tile([128, 256], F32, tag="outt")
            nc.vector.tensor_copy(out=outt[:], in_=po2[:])
            nc.sync.dma_start(out=p_out[b * BLK:(b + 1) * BLK, :], in_=outt[:BLK, :])

    if finalize:
        nc.finalize()
    return nc


def split_waits(nc, keep=1):
    """Walrus caps sync-wait commands per instruction. Move excess waits onto
    EventSemaphore instructions inserted just before, on the same engine."""
    n_split = 0
    for f in nc.m.functions:
        for bb in f.blocks:
            new = []
            changed = False
            for ins in bb.instructions:
                si = ins.sync_info
                tn = type(ins).__name__
                if (si is not None and len(si.on_wait) > keep
                        and tn != "InstEventSemaphore"):
                    waits = list(si.on_wait)
                    k = 0
                    while len(waits) > keep:
                        chunk, waits = waits[:2], waits[2:]
                        es = mybir.InstEventSemaphore(name=f"{ins.name}-ws{k}", ins=[], outs=[])
                        es.engine = ins.engine
                        es.sync_info = mybir.SyncInfo(on_wait=chunk, on_update=[])
                        new.append(es)
                        k += 1
                        n_split += 1
                    ins.sync_info = mybir.SyncInfo(on_wait=waits, on_update=list(si.on_update))
                    changed = True
                new.append(ins)
            if changed:
                bb.instructions = new
    return n_split


_CACHE = {}


def _get_nc(nsub, tblk):
    key = (nsub, tblk)
    if key not in _CACHE:
        _CACHE[key] = build(nsub, tblk)
    return _CACHE[key]


def kernel_run(inputs, trace=False, **kw):
    in_maps, shapes = host_prep(inputs)
    nc = _get_nc(shapes["nsub"], shapes["tblk"])
    res = run_bass_kernel_spmd(nc, in_maps, core_ids=list(range(NCORE)), trace=trace, **kw)
    out = np.concatenate([np.asarray(res.results[c]["out"], np.float32) for c in range(NCORE)], 0)
    return out, res


def kernel(**inputs) -> np.ndarray:
    out, _ = kernel_run(inputs)
    return out



# revision 7
# speedup vs baseline: 2.1805x; 2.1805x over previous
"""Trainium2 8-core Bass kernel for nn_Atom_Inter_Layer (GNN attention message passing).

Strategy: edges sharded by destination-node range (core c owns nodes
[1250c, 1250(c+1)) and all edges whose dst lands there), so segment
softmax/sum stay core-local — no collectives. Host does index bucketing,
selector-matrix staging and weight folding; all NN compute runs on device.

v2 redesign (vs the exp-decomposed baseline):
  - single ACT table (silu_and_others): mid-LN+SiLU fused into ONE scalar-engine
    Silu op with per-partition scale/bias; score exp synthesized exactly as
    exp(x) = (1+t)/(1-t), t = tanh(x/2) (scores are tiny, |x|<0.3).
  - per-edge 1/std via quake-style rsqrt (bit-trick seed + 1 Newton) batched
    over a whole block on the DVE (7 small ops per 33 subtiles).
  - A'[src] gather-add done on the PE (identity-matmul accumulate onto the
    same PSUM region, start=False) instead of DVE adds.
  - q[dst] folded into the B-table S-matmul (one [v|k|q] 1024-wide PSUM tile).
  - dst selector matrices S (edge-major + node-major) staged from host.
  - layout order [v(512) | k(256) | q(256)] so each matmul out stays in-bank.

Per-core pipeline: prep A'/Bq tables; then per 125-node block: phase A
(L1 matmuls + bn_stats + PSUM->SBUF copy per 128-edge subtile), batched
rsqrt chain, phase B (Silu, PE transposes, L2 matmuls, scores, scatter),
then the alpha-normalize + output-MLP epilogue.
"""
import sys

if "/opt/trn_rl_repo" not in sys.path:
    sys.path.insert(0, "/opt/trn_rl_repo")

from contextlib import ExitStack

import numpy as np
import ml_dtypes

import concourse.bass as bass
import concourse.bacc as bacc
import concourse.tile as tile
import bass_rust as _bass_rust
from concourse.hw_specs import get_activation_tables as _gat


def _patched_iatl(self):
    import concourse.mybir as _mb
    has_activation = any(
        isinstance(i, _mb.InstActivation)
        for b in self.main_func.blocks
        for i in b.instructions
    )
    if not has_activation:
        return
    tables = list(_gat(self.m.arch).items())
    if _FILTER_TABLES:
        keep = "silu_and_others"
        import concourse.mybir as _mb2
        drop = {_mb2.ActivationFunctionType.Silu, _mb2.ActivationFunctionType.Tanh,
                _mb2.ActivationFunctionType.Copy, _mb2.ActivationFunctionType.Identity}
        tables = [(n, (set(fns) if n == keep else {f for f in fns if f not in drop}))
                  for n, fns in tables]
    _bass_rust.insert_act_table_loads(self, tables)


_FILTER_TABLES = True
bacc.Bacc.insert_act_table_loads = _patched_iatl
from concourse import mybir
from concourse.bass_utils import run_bass_kernel_spmd

BF16 = mybir.dt.bfloat16
F32 = mybir.dt.float32
I32 = mybir.dt.int32
I16 = mybir.dt.int16

N, E, D, EDIM, H, C = 10000, 320000, 256, 64, 8, 32
FEAT = 2 * D + EDIM  # 576
NCORE = 8
NB = 10            # node blocks per core
BLK = 125          # nodes per block
NPC = NB * BLK     # nodes per core = 1250
NT = (N + 127) // 128          # 79 tiles of the full node table
NPAD = NT * 128                # 10112
EPS = 1e-5
AW = 1024          # augmented row width: [v 512 | k 256 | q 256]
GCH = 9            # gather chunk (subtiles per dma_gather)
MAGIC = 0x5F3759DF
ISQ = 1.0 / np.sqrt(C)

bf16 = ml_dtypes.bfloat16


def _b(a):
    return np.ascontiguousarray(np.asarray(a, np.float32)).astype(bf16)


def host_prep(inputs):
    """Build per-core in_maps + static shapes from the full inputs."""
    x = np.asarray(inputs["x"], np.float32)
    ei = np.asarray(inputs["edge_index"]).astype(np.int64)
    ea = np.asarray(inputs["edge_attr"], np.float32)
    src, dst = ei[0], ei[1]
    g = np.asarray(inputs["ln_norm_g"], np.float32)
    kw1 = np.asarray(inputs["k_w1"], np.float32)
    vw1 = np.asarray(inputs["v_w1"], np.float32)

    # --- fast-path validity (biases zero / gains one folded trivially) ---
    for nm in ("ln_norm_b", "k_b1", "v_b1", "q_b1", "q_b2", "k_b2", "v_b2",
               "o_b1", "o_b2", "q_be", "k_be", "v_be"):
        assert np.abs(np.asarray(inputs[nm])).max() == 0.0, f"{nm} nonzero; fast path invalid"
    for nm in ("q_g", "k_g", "v_g"):
        assert np.abs(np.asarray(inputs[nm]) - 1.0).max() == 0.0, f"{nm} != 1"

    # --- weight folds (f32 host math); output order [v(512) | k(256)] ---
    v1cat = np.concatenate([g @ vw1, g @ kw1])                     # [768]
    rk1 = v1cat[None, :] / FEAT   # rank-1 LN-mean fold
    gk = g[:, None]
    wEA = np.concatenate([gk[:64] * vw1[0:64], gk[:64] * kw1[0:64]], 1) - rk1      # [64,768]
    wA = np.concatenate([gk[64:320] * vw1[64:320], gk[64:320] * kw1[64:320]], 1) - rk1
    wB = np.concatenate([gk[320:576] * vw1[320:576], gk[320:576] * kw1[320:576]], 1) - rk1
    wEAf = np.zeros((EDIM, AW), np.float32)
    wEAf[:, 0:768] = wEA
    wAf = np.zeros((D, AW), np.float32)
    wAf[:, 0:768] = wA

    # --- edge bucketing by destination block ---
    bucket = (dst // BLK).astype(np.int64)           # 0..79
    order = np.argsort(bucket, kind="stable")
    counts = np.bincount(bucket, minlength=NCORE * NB)
    nsub = int(np.ceil(counts.max() / 128))
    tblk = nsub * 128
    starts = np.zeros(NCORE * NB, np.int64)
    starts[1:] = np.cumsum(counts)[:-1]
    pos_in_blk = np.arange(E, dtype=np.int64) - starts[bucket[order]]

    idx_pad = np.zeros((NCORE * NB, tblk), np.int32)          # src gather index (pad -> 0)
    dst_pad = np.full((NCORE * NB, tblk), -1.0, np.float32)   # block-local dst (pad -> -1)
    ea_pad = np.zeros((NCORE * NB, tblk, EDIM), np.float32)
    bo = bucket[order]
    idx_pad[bo, pos_in_blk] = src[order].astype(np.int32)
    dst_pad[bo, pos_in_blk] = (dst[order] - bo * BLK).astype(np.float32)
    ea_pad[bo, pos_in_blk, :] = ea[order]

    # eaT: [core][64, NB*tblk], column order (block, t, p)
    eaT = ea_pad.reshape(NCORE, NB * tblk, EDIM).transpose(0, 2, 1)
    # dma_gather int16 indices: idx i at [i%16, i//16], replicated to 128 partitions
    idx16 = idx_pad.astype(np.int16).reshape(NCORE, NB, tblk // 16, 16).transpose(0, 1, 3, 2)
    idx16 = np.broadcast_to(idx16[:, :, None, :, :], (NCORE, NB, 8, 16, tblk // 16))
    idx16 = np.ascontiguousarray(idx16).reshape(NCORE, NB, 128, tblk // 16)

    # selector matrices: S[c,b,t,p,n] = (dst_local(edge p of subtile t) == n)
    dstb = dst_pad.reshape(NCORE, NB, nsub, 128)
    S = (dstb[..., None] == np.arange(128, dtype=np.float32)).astype(bf16)
    Seb = np.ascontiguousarray(S.transpose(0, 1, 3, 2, 4))   # [c, b, p(edge), t, n]
    Sne = np.ascontiguousarray(S.transpose(0, 1, 4, 2, 3))   # [c, b, n, t, p(edge)]

    # node-table layouts
    xpad = np.zeros((NPAD, D), np.float32)
    xpad[:N] = x
    xTfull = _b(xpad.T.reshape(D, NPAD))                    # [256, 10112]
    xTblk = x.reshape(NCORE, NB, BLK, D)
    xTb = np.zeros((NCORE, D, NB, 128), np.float32)
    xTb[:, :, :, :BLK] = xTblk.transpose(0, 3, 1, 2)

    ident = np.eye(128, dtype=np.float32)

    # wkv2: chunks 0..3 act on v-hidden -> out cols 256:512; 4..5 on k-hidden -> 0:256
    wkv2 = np.concatenate([
        np.asarray(inputs["v_w2"], np.float32).reshape(4, 128, 256),
        np.asarray(inputs["k_w2"], np.float32).reshape(2, 128, 256),
    ], 0)                                                    # [6,128,256]

    shapes = dict(nsub=nsub, tblk=tblk)
    common = {
        "wEAx": _b(wEAf),
        "wA": _b(wAf.reshape(2, 128, AW)),
        "wB": _b(wB.reshape(2, 128, 768)),
        "wq1": _b(np.asarray(inputs["q_w1"], np.float32).reshape(2, 128, 512)),
        "wq2": _b(np.asarray(inputs["q_w2"], np.float32).reshape(4, 128, 256)),
        "wkv2": _b(wkv2),
        "wo1": _b(np.asarray(inputs["o_w1"], np.float32).reshape(2, 128, 512)),
        "wo2": _b(np.asarray(inputs["o_w2"], np.float32).reshape(4, 128, 256)),
        "ident": _b(ident),
        "xTfull": xTfull,
    }
    in_maps = []
    for c in range(NCORE):
        m = dict(common)
        m["eaT"] = _b(eaT[c])
        m["idx"] = np.ascontiguousarray(idx16[c])
        m["Seb"] = np.ascontiguousarray(Seb[c])
        m["Sne"] = np.ascontiguousarray(Sne[c])
        m["xTb"] = _b(xTb[c].reshape(D, NB * 128))
        in_maps.append(m)
    return in_maps, shapes


def build(nsub, tblk, debug=False, finalize=True):
    """Build the single-core Bass graph (same on all 8 cores)."""
    nc = bacc.Bacc()
    p_eaT = nc.declare_dram_parameter("eaT", [EDIM, NB * tblk], BF16, isOutput=False)
    p_idx = nc.declare_dram_parameter("idx", [NB, 128, tblk // 16], I16, isOutput=False)
    p_Seb = nc.declare_dram_parameter("Seb", [NB, 128, nsub, 128], BF16, isOutput=False)
    p_Sne = nc.declare_dram_parameter("Sne", [NB, 128, nsub, 128], BF16, isOutput=False)
    p_xTb = nc.declare_dram_parameter("xTb", [D, NB * 128], BF16, isOutput=False)
    p_xTf = nc.declare_dram_parameter("xTfull", [D, NPAD], BF16, isOutput=False)
    p_wEAx = nc.declare_dram_parameter("wEAx", [EDIM, AW], BF16, isOutput=False)
    p_wA = nc.declare_dram_parameter("wA", [2, 128, AW], BF16, isOutput=False)
    p_wB = nc.declare_dram_parameter("wB", [2, 128, 768], BF16, isOutput=False)
    p_wq1 = nc.declare_dram_parameter("wq1", [2, 128, 512], BF16, isOutput=False)
    p_wq2 = nc.declare_dram_parameter("wq2", [4, 128, 256], BF16, isOutput=False)
    p_wkv2 = nc.declare_dram_parameter("wkv2", [6, 128, 256], BF16, isOutput=False)
    p_wo1 = nc.declare_dram_parameter("wo1", [2, 128, 512], BF16, isOutput=False)
    p_wo2 = nc.declare_dram_parameter("wo2", [4, 128, 256], BF16, isOutput=False)
    p_ident = nc.declare_dram_parameter("ident", [128, 128], BF16, isOutput=False)
    p_out = nc.declare_dram_parameter("out", [NPC, D], F32, isOutput=True)
    p_dbg = nc.declare_dram_parameter("dbg", [8, 128, AW], F32, isOutput=True) if debug else None
    A_dram = nc.dram_tensor("A_tab", [NPAD, AW], BF16)

    with tile.TileContext(nc) as tc, ExitStack() as ctx:
        const = ctx.enter_context(tc.tile_pool(name="const", bufs=1))
        persist = ctx.enter_context(tc.tile_pool(name="persist", bufs=1))
        # psum pools: ppT 2x2 banks + ppKV 2x1 + ppY 1 + ppA 1 = 8 banks
        ppT = ctx.enter_context(tc.tile_pool(name="ppT", bufs=2, space="PSUM"))
        ppKV = ctx.enter_context(tc.tile_pool(name="ppKV", bufs=2, space="PSUM"))
        ppY = ctx.enter_context(tc.tile_pool(name="ppY", bufs=1, space="PSUM"))
        ppA = ctx.enter_context(tc.tile_pool(name="ppA", bufs=1, space="PSUM"))
        # sbuf pools
        sp_g = ctx.enter_context(tc.tile_pool(name="sp_g", bufs=2))      # gather chunks
        sp_blk = ctx.enter_context(tc.tile_pool(name="sp_blk", bufs=2))  # per-block loads
        sp_h = ctx.enter_context(tc.tile_pool(name="sp_h", bufs=1))      # h1q per block
        sp_s = ctx.enter_context(tc.tile_pool(name="sp_s", bufs=3))      # s / sT tiles
        sp_w = ctx.enter_context(tc.tile_pool(name="sp_w", bufs=3))      # prod / m_ext
        sp_t = ctx.enter_context(tc.tile_pool(name="sp_t", bufs=4))      # small f32
        sp_c = ctx.enter_context(tc.tile_pool(name="sp_c", bufs=2))      # chain tiles
        sp_o = ctx.enter_context(tc.tile_pool(name="sp_o", bufs=2))      # outputs / A rows

        def cload(param, shape, dtype=BF16, rearr=None, **rkw):
            t = const.tile(shape, dtype, tag=param.name)
            src = param[:]
            if rearr:
                src = src.rearrange(rearr, **rkw)
            nc.sync.dma_start(out=t[:], in_=src)
            return t

        wEAx = cload(p_wEAx, [EDIM, AW])
        wA = cload(p_wA, [128, 2, AW], rearr="j p c -> p j c")
        wB = cload(p_wB, [128, 2, 768], rearr="j p c -> p j c")
        wq1 = cload(p_wq1, [128, 2, 512], rearr="j p c -> p j c")
        wq2 = cload(p_wq2, [128, 4, 256], rearr="j p c -> p j c")
        wkv2 = cload(p_wkv2, [128, 6, 256], rearr="j p c -> p j c")
        wo1 = cload(p_wo1, [128, 2, 512], rearr="j p c -> p j c")
        wo2 = cload(p_wo2, [128, 4, 256], rearr="j p c -> p j c")
        ident = cload(p_ident, [128, 128])
        xTb = cload(p_xTb, [128, 2, NB * 128], rearr="(j p) n -> p j n", p=128)

        Bq_sb = persist.tile([128, NB, AW], BF16)

        def rsqrt_chain(mean_ap, var_ap, nlane, rs_t, b2_t):
            """rs = rsqrt(var+eps), b2 = -mean*rs via quake seed + 1 Newton.
            mean_ap/var_ap: [128, nlane] f32 APs; rs_t/b2_t: packed [128, nlane]."""
            ve = sp_c.tile([128, nlane], F32, tag="ve")
            nc.vector.tensor_scalar(out=ve[:], in0=var_ap, scalar1=EPS, scalar2=None,
                                    op0=mybir.AluOpType.add)
            t1 = sp_c.tile([128, nlane], I32, tag="t1")
            nc.vector.tensor_scalar(out=t1[:], in0=ve[:].bitcast(I32), scalar1=1,
                                    scalar2=None, op0=mybir.AluOpType.arith_shift_right)
            y0 = sp_c.tile([128, nlane], I32, tag="y0")
            nc.vector.tensor_scalar(out=y0[:], in0=t1[:], scalar1=-1, scalar2=MAGIC,
                                    op0=mybir.AluOpType.mult, op1=mybir.AluOpType.add)
            y0f = y0[:].bitcast(F32)
            p = sp_c.tile([128, nlane], F32, tag="p")
            nc.vector.tensor_tensor(out=p[:], in0=y0f, in1=y0f, op=mybir.AluOpType.mult)
            qq = sp_c.tile([128, nlane], F32, tag="qq")
            nc.vector.scalar_tensor_tensor(out=qq[:], in0=ve[:], scalar=-0.5, in1=p[:],
                                           op0=mybir.AluOpType.mult, op1=mybir.AluOpType.mult)
            nc.vector.scalar_tensor_tensor(out=rs_t[:], in0=qq[:], scalar=1.5, in1=y0f,
                                           op0=mybir.AluOpType.add, op1=mybir.AluOpType.mult)
            nc.vector.scalar_tensor_tensor(out=b2_t[:], in0=mean_ap, scalar=-1.0, in1=rs_t[:],
                                           op0=mybir.AluOpType.mult, op1=mybir.AluOpType.mult)

        # ================= PREP =================
        prep_stack = ExitStack()
        prepc = prep_stack.enter_context(tc.tile_pool(name="prepc", bufs=3))
        for i in range(NT):
            xTf = prepc.tile([128, 2, 128], BF16, tag="xTf")
            nc.sync.dma_start(out=xTf[:],
                              in_=p_xTf[:, i * 128:(i + 1) * 128]
                              .rearrange("(j p) n -> p j n", p=128))
            T = ppT.tile([128, AW], F32, tag="T")
            for j in range(2):
                nc.tensor.matmul(T[:, 0:512], xTf[:, j, :], wA[:, j, 0:512],
                                 start=(j == 0), stop=(j == 1))
            for j in range(2):
                nc.tensor.matmul(T[:, 512:1024], xTf[:, j, :], wA[:, j, 512:1024],
                                 start=(j == 0), stop=(j == 1))
            at = sp_o.tile([128, AW], BF16, tag="atab")
            nc.scalar.copy(out=at[:, 0:512], in_=T[:, 0:512])
            nc.vector.tensor_copy(out=at[:, 512:1024], in_=T[:, 512:1024])
            nc.gpsimd.dma_start(out=A_dram[i * 128:(i + 1) * 128, :], in_=at[:])

        for b in range(NB):
            # B' part
            T2 = ppT.tile([128, AW], F32, tag="T")
            for j in range(2):
                lhsT = xTb[:, j, b * 128:(b + 1) * 128]
                nc.tensor.matmul(T2[:, 0:512], lhsT, wB[:, j, 0:512],
                                 start=(j == 0), stop=(j == 1))
            for j in range(2):
                lhsT = xTb[:, j, b * 128:(b + 1) * 128]
                nc.tensor.matmul(T2[:, 512:768], lhsT, wB[:, j, 512:768],
                                 start=(j == 0), stop=(j == 1))
            nc.scalar.copy(out=Bq_sb[:, b, 0:512], in_=T2[:, 0:512])
            nc.vector.tensor_copy(out=Bq_sb[:, b, 512:768], in_=T2[:, 512:768])
            # q part: Linear -> LN -> SiLU -> Linear
            pq1 = ppKV.tile([128, 512], F32, tag="kv")
            for j in range(2):
                nc.tensor.matmul(pq1[:], xTb[:, j, b * 128:(b + 1) * 128], wq1[:, j, :],
                                 start=(j == 0), stop=(j == 1))
            stq = sp_t.tile([128, 6], F32, tag="stq")
            nc.vector.bn_stats(out=stq[:], in_=pq1[:])
            mvq = sp_t.tile([128, 2], F32, tag="mvq")
            nc.vector.bn_aggr(out=mvq[:], in_=stq[:])
            rsq = sp_c.tile([128, 1], F32, tag="rs")
            b2q = sp_c.tile([128, 1], F32, tag="b2")
            rsqrt_chain(mvq[:, 0:1], mvq[:, 1:2], 1, rsq, b2q)
            sq = sp_s.tile([128, 768], BF16, tag="s")
            nc.scalar.activation(out=sq[:, 0:512], in_=pq1[:],
                                 func=mybir.ActivationFunctionType.Silu,
                                 bias=b2q[:], scale=rsq[:])
            yT = ppY.tile([128, 768], BF16, tag="yT")
            for i in range(4):
                nc.tensor.transpose(yT[:, i * 128:(i + 1) * 128],
                                    sq[:, i * 128:(i + 1) * 128], ident[:])
            sqT = sp_s.tile([128, 768], BF16, tag="sT")
            nc.vector.tensor_copy(out=sqT[:, 0:512], in_=yT[:, 0:512])
            pq2 = ppKV.tile([128, 512], F32, tag="kv")
            for i in range(4):
                nc.tensor.matmul(pq2[:, 0:256], sqT[:, i * 128:(i + 1) * 128], wq2[:, i, :],
                                 start=(i == 0), stop=(i == 3))
            nc.scalar.copy(out=Bq_sb[:, b, 768:1024], in_=pq2[:, 0:256])

        prep_stack.close()
        tc.strict_bb_all_engine_barrier()

        # ================= MAIN =================
        chunks = [(s, min(s + GCH, nsub)) for s in range(0, nsub, GCH)]

        for b in range(NB):
            idx_t = sp_blk.tile([128, tblk // 16], I16, tag="idx")
            nc.sync.dma_start(out=idx_t[:], in_=p_idx[b])

            mv_all = sp_c.tile([128, nsub, 4], F32, tag="mv")
            h1q = sp_h.tile([128, nsub, AW], BF16, tag="h1q")
            acc = ppA.tile([128, 264], F32, tag="acc")

            # ---- phase A: L1 matmuls + stats + PSUM->SBUF ----
            for (h0, h1c) in chunks:
                cnt = h1c - h0
                ag = sp_g.tile([128, GCH, AW], BF16, tag="ag")
                nc.gpsimd.dma_gather(
                    out_ap=ag[:, 0:cnt, :],
                    in_ap=A_dram[:],
                    idxs_ap=idx_t[:, h0 * 8:h1c * 8],
                    num_idxs=cnt * 128,
                    num_idxs_reg=cnt * 128,
                    elem_size=AW,
                    single_packet=False,
                )
                eaT_t = sp_g.tile([EDIM, GCH * 128], BF16, tag="ea")
                nc.sync.dma_start(out=eaT_t[:, 0:cnt * 128],
                                  in_=p_eaT[:, b * tblk + h0 * 128:b * tblk + h1c * 128])
                Sne_t = sp_g.tile([128, GCH, 128], BF16, tag="Sne")
                nc.sync.dma_start(out=Sne_t[:, 0:cnt, :], in_=p_Sne[b, :, h0:h1c, :])
                for t in range(h0, h1c):
                    j = t - h0
                    T = ppT.tile([128, AW], F32, tag="T")
                    ea_l = eaT_t[:, j * 128:(j + 1) * 128]
                    S_ne = Sne_t[:, j, :]
                    nc.tensor.matmul(T[:, 0:512], ea_l, wEAx[:, 0:512],
                                     start=True, stop=False)
                    nc.tensor.matmul(T[:, 512:1024], ea_l, wEAx[:, 512:1024],
                                     start=True, stop=False)
                    nc.tensor.matmul(T[:, 0:512], S_ne, Bq_sb[:, b, 0:512],
                                     start=False, stop=False)
                    nc.tensor.matmul(T[:, 512:1024], S_ne, Bq_sb[:, b, 512:1024],
                                     start=False, stop=False)
                    nc.tensor.matmul(T[:, 0:512], ident[:], ag[:, j, 0:512],
                                     start=False, stop=True)
                    nc.tensor.matmul(T[:, 512:1024], ident[:], ag[:, j, 512:1024],
                                     start=False, stop=True)
                    st6 = sp_t.tile([128, 2, 6], F32, tag="st6")
                    nc.vector.bn_stats(out=st6[:, 0, :], in_=T[:, 0:512])
                    nc.vector.bn_aggr(out=mv_all[:, t, 0:2], in_=st6[:, 0, :])
                    nc.vector.bn_stats(out=st6[:, 1, :], in_=T[:, 512:768])
                    nc.vector.bn_aggr(out=mv_all[:, t, 2:4], in_=st6[:, 1, :])
                    nc.scalar.copy(out=h1q[:, t, 0:512], in_=T[:, 0:512])
                    nc.scalar.copy(out=h1q[:, t, 512:1024], in_=T[:, 512:1024])

            # ---- rsqrt chain, batched over the block ----
            mvap = mv_all[:]
            mean_ap = bass.AP(tensor=mvap.tensor, offset=mvap.offset,
                              ap=[mvap.ap[0], [4, nsub], [2, 2]])
            var_ap = bass.AP(tensor=mvap.tensor, offset=mvap.offset + 1,
                             ap=[mvap.ap[0], [4, nsub], [2, 2]])
            rs_all = sp_c.tile([128, nsub, 2], F32, tag="rsa")
            b2_all = sp_c.tile([128, nsub, 2], F32, tag="b2a")
            rsqrt_chain(mean_ap, var_ap, nsub * 2,
                        rs_all, b2_all)

            # ---- phase B: SiLU + transposes + L2 + scores + scatter ----
            for (h0, h1c) in chunks:
                cnt = h1c - h0
                Seb_t = sp_blk.tile([128, GCH, 128], BF16, tag="Seb")
                nc.sync.dma_start(out=Seb_t[:, 0:cnt, :], in_=p_Seb[b, :, h0:h1c, :])
                for t in range(h0, h1c):
                    jj = t - h0
                    s_sb = sp_s.tile([128, 768], BF16, tag="s")
                    nc.scalar.activation(out=s_sb[:, 0:512], in_=h1q[:, t, 0:512],
                                         func=mybir.ActivationFunctionType.Silu,
                                         bias=b2_all[:, t, 0:1], scale=rs_all[:, t, 0:1])
                    nc.scalar.activation(out=s_sb[:, 512:768], in_=h1q[:, t, 512:768],
                                         func=mybir.ActivationFunctionType.Silu,
                                         bias=b2_all[:, t, 1:2], scale=rs_all[:, t, 1:2])
                    yT = ppY.tile([128, 768], BF16, tag="yT")
                    for i in range(6):
                        nc.tensor.transpose(yT[:, i * 128:(i + 1) * 128],
                                            s_sb[:, i * 128:(i + 1) * 128], ident[:])
                    sT = sp_s.tile([128, 768], BF16, tag="sT")
                    nc.vector.tensor_copy(out=sT[:], in_=yT[:])
                    kv = ppKV.tile([128, 512], F32, tag="kv")
                    for i in range(4):
                        nc.tensor.matmul(kv[:, 256:512], sT[:, i * 128:(i + 1) * 128],
                                         wkv2[:, i, :], start=(i == 0), stop=(i == 3))
                    for i in range(2):
                        nc.tensor.matmul(kv[:, 0:256], sT[:, (4 + i) * 128:(5 + i) * 128],
                                         wkv2[:, 4 + i, :], start=(i == 0), stop=(i == 1))
                    prod = sp_w.tile([128, 8, 32], BF16, tag="prod")
                    nc.vector.tensor_tensor(
                        out=prod[:],
                        in0=kv[:, 0:256].rearrange("p (h c) -> p h c", h=8),
                        in1=h1q[:, t, 768:1024].rearrange("p (h c) -> p h c", h=8),
                        op=mybir.AluOpType.mult)
                    sc = sp_t.tile([128, 8], F32, tag="sc")
                    nc.vector.tensor_reduce(out=sc[:], in_=prod[:],
                                            axis=mybir.AxisListType.X, op=mybir.AluOpType.add)
                    th = sp_t.tile([128, 8], F32, tag="th")
                    nc.scalar.activation(out=th[:], in_=sc[:],
                                         func=mybir.ActivationFunctionType.Tanh,
                                         bias=0.0, scale=float(ISQ * 0.5))
                    am = sp_t.tile([128, 8], F32, tag="am")
                    nc.vector.tensor_scalar(out=am[:], in0=th[:], scalar1=-1.0, scalar2=1.0,
                                            op0=mybir.AluOpType.mult, op1=mybir.AluOpType.add)
                    rm = sp_t.tile([128, 8], F32, tag="rm")
                    nc.vector.reciprocal_approx_fast(out=rm[:], in_=am[:])
                    m_ext = sp_w.tile([128, 264], BF16, tag="mext")
                    nc.vector.scalar_tensor_tensor(out=m_ext[:, 256:264], in0=th[:], scalar=1.0,
                                                   in1=rm[:], op0=mybir.AluOpType.add,
                                                   op1=mybir.AluOpType.mult)
                    u = m_ext[:, 256:264]
                    ubc = bass.AP(tensor=u.tensor, offset=u.offset,
                                  ap=[u.ap[0], u.ap[1], [0, 32]])
                    nc.vector.tensor_tensor(
                        out=m_ext[:, 0:256].rearrange("p (h c) -> p h c", h=8),
                        in0=kv[:, 256:512].rearrange("p (h c) -> p h c", h=8),
                        in1=ubc, op=mybir.AluOpType.mult)
                    nc.tensor.matmul(acc[:], Seb_t[:, jj, :], m_ext[:],
                                     start=(t == 0), stop=(t == nsub - 1))

                    if debug and b == 0 and t == 0:
                        dT = sp_o.tile([128, AW], F32, tag="dbgT")
                        nc.vector.tensor_copy(out=dT[:, 0:512], in_=kv[:])
                        nc.gpsimd.dma_start(out=p_dbg[0, :, 0:512], in_=dT[:, 0:512])
                        nc.gpsimd.dma_start(out=p_dbg[1, :, 0:768], in_=s_sb[:])
                        nc.gpsimd.dma_start(out=p_dbg[2], in_=h1q[:, 0, :])
                        nc.gpsimd.dma_start(out=p_dbg[3, :, 0:264], in_=m_ext[:])
                        dc = sp_o.tile([128, AW], F32, tag="dbgT")
                        nc.vector.tensor_copy(out=dc[:, 0:2], in_=rs_all[:, 0, :])
                        nc.vector.tensor_copy(out=dc[:, 2:4], in_=b2_all[:, 0, :])
                        nc.vector.tensor_copy(out=dc[:, 4:8], in_=mv_all[:, 0, :])
                        nc.gpsimd.dma_start(out=p_dbg[4, :, 0:8], in_=dc[:, 0:8])

            # ---- block epilogue: alpha-normalize + output MLP ----
            dmx = sp_t.tile([128, 8], F32, tag="dmx")
            nc.vector.tensor_scalar_max(out=dmx[:], in0=acc[:, 256:264], scalar1=1e-30)
            rec = sp_t.tile([128, 8], F32, tag="rec")
            nc.vector.reciprocal_approx_fast(out=rec[:], in_=dmx[:])
            agg = sp_w.tile([128, 256], BF16, tag="agg")
            rap = rec[:]
            rbc = bass.AP(tensor=rap.tensor, offset=rap.offset,
                          ap=[rap.ap[0], rap.ap[1], [0, 32]])
            nc.vector.tensor_tensor(out=agg[:].rearrange("p (h c) -> p h c", h=8),
                                    in0=acc[:, 0:256].rearrange("p (h c) -> p h c", h=8),
                                    in1=rbc, op=mybir.AluOpType.mult)
            yT2 = ppY.tile([128, 768], BF16, tag="yT")
            for i in range(2):
                nc.tensor.transpose(yT2[:, i * 128:(i + 1) * 128],
                                    agg[:, i * 128:(i + 1) * 128], ident[:])
            aT = sp_s.tile([128, 768], BF16, tag="sT")
            nc.vector.tensor_copy(out=aT[:, 0:256], in_=yT2[:, 0:256])
            po1 = ppKV.tile([128, 512], F32, tag="kv")
            for i in range(2):
                nc.tensor.matmul(po1[:], aT[:, i * 128:(i + 1) * 128], wo1[:, i, :],
                                 start=(i == 0), stop=(i == 1))
            so = sp_s.tile([128, 768], BF16, tag="s")
            nc.scalar.activation(out=so[:, 0:512], in_=po1[:],
                                 func=mybir.ActivationFunctionType.Silu,
                                 bias=0.0, scale=1.0)
            yT3 = ppY.tile([128, 768], BF16, tag="yT")
            for i in range(4):
                nc.tensor.transpose(yT3[:, i * 128:(i + 1) * 128],
                                    so[:, i * 128:(i + 1) * 128], ident[:])
            soT = sp_s.tile([128, 768], BF16, tag="sT")
            nc.vector.tensor_copy(out=soT[:, 0:512], in_=yT3[:, 0:512])
            po2 = ppKV.tile([128, 512], F32, tag="kv")
            for i in range(4):
                nc.tensor.matmul(po2[:, 0:256], soT[:, i * 128:(i + 1) * 128], wo2[:, i, :],
                                 start=(i == 0), stop=(i == 3))
            outt = sp_o.tile([128, 256], F32, tag="outt")
            nc.scalar.copy(out=outt[:], in_=po2[:, 0:256])
            nc.sync.dma_start(out=p_out[b * BLK:(b + 1) * BLK, :], in_=outt[:BLK, :])

    if finalize:
        nc.finalize()
    return nc


_CACHE = {}


def _get_nc(nsub, tblk):
    key = (nsub, tblk)
    if key not in _CACHE:
        _CACHE[key] = build(nsub, tblk)
    return _CACHE[key]


def kernel_run(inputs, trace=False, **kw):
    in_maps, shapes = host_prep(inputs)
    nc = _get_nc(shapes["nsub"], shapes["tblk"])
    res = run_bass_kernel_spmd(nc, in_maps, core_ids=list(range(NCORE)), trace=trace, **kw)
    out = np.concatenate([np.asarray(res.results[c]["out"], np.float32) for c in range(NCORE)], 0)
    return out, res


def kernel(**inputs) -> np.ndarray:
    out, _ = kernel_run(inputs)
    return out
